# revision 3
# baseline (speedup 1.0000x reference)
"""GCNConv layer on 8 Trainium2 NeuronCores (Bass/Tile).

Strategy (graph/data parallel, edges partitioned by destination):
  out = relu( D^-1/2 (A+I) D^-1/2 (x W) + b ) + x
      = relu( (dinv_d * (sum_{e->d} dinv_s x_s + dinv_d x_d)) @ W + b ) + x
(using linearity: the W matmul is applied after aggregation).

Each core owns N/8 destination nodes. Per core:
  - source nodes are split into 4 chunks of N/4 rows so gather indices fit
    int16 (dma_gather requirement)
  - per (chunk c, dst-half h): destinations ordered by in-degree from chunk c
    (descending), so the k-th incoming edge of every dst forms a *prefix* of
    the ordering (ELL layout).  Pass k = one dma_gather of the k-th edges'
    source rows + DVE multiply by dinv_src + DVE accumulate.
  - per-phase accumulator merged into an HBM `agg` table with dma_scatter_add
    (distinct indices; merges chained to avoid RMW races)
  - agg is initialized with the self-loop term dinv_d * x_d
  - final phase: load agg, scale by dinv_d, transpose via PE, matmul with W,
    fused bias+relu on ACT, transpose back, add residual x, store.

Edge sets are padded with weight-0 fake edges so all 8 cores run the exact
same static program (SPMD) with per-core data only.
"""

import sys
import types

sys.path.insert(0, "/opt/trn_rl_repo")

import numpy as np

DIM = 64
N_CORES = 8
N_CHUNKS = 4
N_HALVES = 2
P = 128


def _install_ntff_hook():
    """run_bass_kernel_spmd(trace=True) needs antenv.axon_hooks; the image
    lacks it - install an equivalent backed by libaxon_pjrt.so."""
    if "antenv.axon_hooks" in sys.modules:
        return
    try:
        sys.path.insert(0, "/root/.axon_site")
        from trn_agent_boot.trn_boot import _ntff_profile_via_ctypes

        hook = _ntff_profile_via_ctypes("/opt/axon/libaxon_pjrt.so")
    except Exception:
        hook = None
    mod = types.ModuleType("antenv.axon_hooks")
    mod.get_axon_ntff_profile_hook = lambda: hook
    mod.set_axon_ntff_profile_hook = lambda h: None
    sys.modules["antenv.axon_hooks"] = mod


class Plan:
    """Static (core-independent) program structure."""

    def __init__(self, n_nodes, n_cores, n_chunks, n_halves):
        assert n_nodes % n_cores == 0
        assert n_nodes % n_chunks == 0
        self.N = n_nodes
        self.n_cores = n_cores
        self.n_chunks = n_chunks
        self.n_halves = n_halves
        self.SHARD = n_nodes // n_cores          # dst rows per core
        self.CH = n_nodes // n_chunks            # src rows per chunk
        assert self.CH <= 32767, "chunk must fit int16 index"
        assert self.SHARD % n_halves == 0
        self.HALF = self.SHARD // n_halves       # dst rows per phase
        self.SHB = -(-self.SHARD // P)           # shard blocks (ceil)
        self.ACCB = -(-self.HALF // P)           # accumulator blocks
        self.ACC_SLOTS = self.ACCB * P
        assert self.ACC_SLOTS % 16 == 0
        self.MCOLS = self.ACC_SLOTS // 16
        self.n_phases = n_chunks * n_halves
        self.pass_sizes = None   # [phase][k] -> padded slot count (all cores)
        self.g16_off = None      # [phase][k] -> col offset into gidx blob
        self.g128_off = None     # [phase][k] -> col offset into gwgt blob
        self.GCOLS = 0
        self.WCOLS = 0


def _rep16(vals_i16, n):
    """[n] int -> [128, n//16] wrapped-in-16-partitions, replicated 8x."""
    a = np.asarray(vals_i16, dtype=np.int16).reshape(n // 16, 16).T  # [16, n/16]
    return np.tile(a, (8, 1))


def preprocess(x, edge_index, W, b):
    """Host-side sharding: build the static plan + per-core input maps."""
    x = np.ascontiguousarray(np.asarray(x, dtype=np.float32))
    N = x.shape[0]
    plan = Plan(N, N_CORES, N_CHUNKS, N_HALVES)
    src = np.asarray(edge_index[0], dtype=np.int64)
    dst = np.asarray(edge_index[1], dtype=np.int64)
    deg = np.bincount(dst, minlength=N).astype(np.float64) + 1.0
    dinv = (1.0 / np.sqrt(deg)).astype(np.float32)

    SHARD, CH, HALF = plan.SHARD, plan.CH, plan.HALF
    NPH = plan.n_phases

    core_of = dst // SHARD
    per_core = []  # [core][phase] -> dict(passes=[src arrays], merge_idx)
    for i in range(N_CORES):
        m = core_of == i
        s_i = src[m]
        d_i = dst[m] - i * SHARD
        c_i = s_i // CH
        h_i = d_i // HALF
        phases = []
        for c in range(N_CHUNKS):
            for h in range(N_HALVES):
                mm = (c_i == c) & (h_i == h)
                s = s_i[mm]
                d = d_i[mm] - h * HALF
                deg_ch = np.bincount(d, minlength=HALF)
                order = np.argsort(-deg_ch, kind="stable")  # rank -> dst slot
                rank = np.empty(HALF, dtype=np.int64)
                rank[order] = np.arange(HALF)
                perm = np.argsort(rank[d], kind="stable")
                s_sorted = s[perm]
                counts = deg_ch[order]                  # per rank, descending
                cum = np.concatenate([[0], np.cumsum(counts)])
                K = int(counts[0]) if len(s) else 0
                passes = []
                for k in range(K):
                    L = int(np.searchsorted(-counts, -k, side="left"))
                    pos = cum[:L] + k
                    passes.append(s_sorted[pos])
                merge_idx = order + h * HALF
                phases.append({"passes": passes, "merge_idx": merge_idx})
        per_core.append(phases)

    # static pass structure: max over cores, pad to 128
    pass_sizes = []
    for ph in range(NPH):
        K = max(len(per_core[i][ph]["passes"]) for i in range(N_CORES))
        sizes = []
        for k in range(K):
            L = max(
                len(per_core[i][ph]["passes"][k])
                if k < len(per_core[i][ph]["passes"])
                else 0
                for i in range(N_CORES)
            )
            sizes.append(-(-L // P) * P)
        pass_sizes.append(sizes)
    plan.pass_sizes = pass_sizes

    g16_off, g128_off = [], []
    o16 = o128 = 0
    for ph in range(NPH):
        offs16, offs128 = [], []
        for n in pass_sizes[ph]:
            offs16.append(o16)
            offs128.append(o128)
            o16 += n // 16
            o128 += n // P
        g16_off.append(offs16)
        g128_off.append(offs128)
    plan.g16_off, plan.g128_off = g16_off, g128_off
    plan.GCOLS = max(o16, 16)
    plan.WCOLS = max(o128, 1)

    # per-core blobs
    in_maps = []
    W = np.ascontiguousarray(np.asarray(W, dtype=np.float32))
    b = np.ascontiguousarray(np.asarray(b, dtype=np.float32).reshape(DIM, 1))
    for i in range(N_CORES):
        gidx = np.zeros((P, plan.GCOLS), dtype=np.int16)
        gwgt = np.zeros((P, plan.WCOLS), dtype=np.float32)
        midx = np.full((P, NPH * plan.MCOLS), -1, dtype=np.int16)
        for ph in range(NPH):
            c = ph // N_HALVES
            pdata = per_core[i][ph]
            for k, n in enumerate(pass_sizes[ph]):
                s_pass = (
                    pdata["passes"][k]
                    if k < len(pdata["passes"])
                    else np.empty(0, np.int64)
                )
                L = len(s_pass)
                iv = np.zeros(n, dtype=np.int16)
                wv = np.zeros(n, dtype=np.float32)
                iv[:L] = (s_pass - c * CH).astype(np.int16)
                wv[:L] = dinv[s_pass]
                gidx[:, g16_off[ph][k] : g16_off[ph][k] + n // 16] = _rep16(iv, n)
                gwgt[:, g128_off[ph][k] : g128_off[ph][k] + n // P] = (
                    wv.reshape(n // P, P).T
                )
            mv = np.full(plan.ACC_SLOTS, -1, dtype=np.int16)
            mv[:HALF] = pdata["merge_idx"].astype(np.int16)
            midx[:, ph * plan.MCOLS : (ph + 1) * plan.MCOLS] = _rep16(
                mv, plan.ACC_SLOTS
            )
        dv = np.zeros((plan.SHB * P,), dtype=np.float32)
        dv[:SHARD] = dinv[i * SHARD : (i + 1) * SHARD]
        dinvd = np.ascontiguousarray(dv.reshape(plan.SHB, P).T)  # [p,b]=dinv[b*128+p]
        xsh = np.ascontiguousarray(x[i * SHARD : (i + 1) * SHARD])
        in_maps.append(
            {
                "x": x,
                "xsh": xsh,
                "w": W,
                "bias": b,
                "dinvd": dinvd,
                "gidx": gidx,
                "gwgt": gwgt,
                "midx": midx,
            }
        )
    return plan, in_maps


def build_program(plan):
    from concourse import bacc, mybir
    import concourse.tile as tile
    from concourse.masks import make_identity
    from concourse.tile import add_dep_helper

    N = plan.N
    SHARD, CH, HALF = plan.SHARD, plan.CH, plan.HALF
    SHB, ACCB, MCOLS = plan.SHB, plan.ACCB, plan.MCOLS
    NPH = plan.n_phases
    FB = SHARD // P              # full shard blocks
    REM = SHARD - FB * P         # partial block rows
    f32 = mybir.dt.float32
    i16 = mybir.dt.int16
    mult = mybir.AluOpType.mult
    add = mybir.AluOpType.add

    nc = bacc.Bacc("TRN2", target_bir_lowering=False)
    x_d = nc.dram_tensor("x", [N, DIM], f32, kind="ExternalInput")
    xsh_d = nc.dram_tensor("xsh", [SHARD, DIM], f32, kind="ExternalInput")
    w_d = nc.dram_tensor("w", [DIM, DIM], f32, kind="ExternalInput")
    b_d = nc.dram_tensor("bias", [DIM, 1], f32, kind="ExternalInput")
    dinvd_d = nc.dram_tensor("dinvd", [P, SHB], f32, kind="ExternalInput")
    gidx_d = nc.dram_tensor("gidx", [P, plan.GCOLS], i16, kind="ExternalInput")
    gwgt_d = nc.dram_tensor("gwgt", [P, plan.WCOLS], f32, kind="ExternalInput")
    midx_d = nc.dram_tensor("midx", [P, NPH * MCOLS], i16, kind="ExternalInput")
    agg_d = nc.dram_tensor("agg", [SHB * P, DIM], f32)
    out_d = nc.dram_tensor("out", [SHARD, DIM], f32, kind="ExternalOutput")

    max_nblk = max((max(s) for s in plan.pass_sizes if s), default=P) // P

    with tile.TileContext(nc) as tc:
        with (
            tc.tile_pool(name="const", bufs=1) as constp,
            tc.tile_pool(name="io", bufs=2) as iop,
            tc.tile_pool(name="gbuf", bufs=3) as gbufp,
            tc.tile_pool(name="accp", bufs=2) as accp,
            tc.tile_pool(name="fin", bufs=3) as finp,
            tc.tile_pool(name="psum", bufs=2, space="PSUM") as psump,
        ):
            ident = constp.tile([P, P], f32)
            make_identity(nc, ident[:])
            w_t = constp.tile([DIM, DIM], f32)
            nc.sync.dma_start(out=w_t[:], in_=w_d[:])
            b_t = constp.tile([DIM, 1], f32)
            nc.sync.dma_start(out=b_t[:], in_=b_d[:])
            dinvd_t = constp.tile([P, SHB], f32)
            nc.sync.dma_start(out=dinvd_t[:], in_=dinvd_d[:])

            # x shard resident: [128, SHB*64], row b*128+p -> [p, b*64:(b+1)*64]
            xs_t = constp.tile([P, SHB * DIM], f32)
            if REM:
                nc.vector.memset(xs_t[:, FB * DIM :], 0.0)
            nc.sync.dma_start(
                out=xs_t[:, : FB * DIM].rearrange("p (bb d) -> p bb d", d=DIM),
                in_=xsh_d[0 : FB * P, :].rearrange("(bb p) d -> p bb d", p=P),
            )
            if REM:
                nc.sync.dma_start(
                    out=xs_t[:REM, FB * DIM :],
                    in_=xsh_d[FB * P : SHARD, :],
                )

            # init agg with self-loop term dinv_d * x_d
            init_t = constp.tile([P, SHB * DIM], f32)
            nc.vector.tensor_tensor(
                out=init_t[:].rearrange("p (bb d) -> p bb d", d=DIM),
                in0=xs_t[:].rearrange("p (bb d) -> p bb d", d=DIM),
                in1=dinvd_t[:].to_broadcast([P, SHB, DIM]),
                op=mult,
            )
            agg_writes = []
            inst = nc.sync.dma_start(
                out=agg_d[0 : FB * P, :].rearrange("(bb p) d -> p bb d", p=P),
                in_=init_t[:, : FB * DIM].rearrange("p (bb d) -> p bb d", d=DIM),
            )
            agg_writes.append(inst)
            if REM:
                inst = nc.sync.dma_start(
                    out=agg_d[FB * P : SHARD, :],
                    in_=init_t[:REM, FB * DIM :],
                )
                agg_writes.append(inst)
                ztail = constp.tile([P - REM, DIM], f32)
                nc.vector.memset(ztail[:], 0.0)
                inst = nc.sync.dma_start(out=agg_d[SHARD:, :], in_=ztail[:])
                agg_writes.append(inst)

            # phases
            merge_insts = []
            for ph in range(NPH):
                c = ph // N_HALVES
                sizes = plan.pass_sizes[ph]
                if sizes:
                    gcols = plan.g16_off[ph][-1] + sizes[-1] // 16 - plan.g16_off[ph][0]
                    wcols = plan.g128_off[ph][-1] + sizes[-1] // P - plan.g128_off[ph][0]
                    gidx_t = iop.tile([P, gcols], i16, tag="gidx")
                    nc.sync.dma_start(
                        out=gidx_t[:],
                        in_=gidx_d[
                            :, plan.g16_off[ph][0] : plan.g16_off[ph][0] + gcols
                        ],
                    )
                    gwgt_t = iop.tile([P, wcols], f32, tag="gwgt")
                    nc.sync.dma_start(
                        out=gwgt_t[:],
                        in_=gwgt_d[
                            :, plan.g128_off[ph][0] : plan.g128_off[ph][0] + wcols
                        ],
                    )
                midx_t = iop.tile([P, MCOLS], i16, tag="midx")
                nc.sync.dma_start(
                    out=midx_t[:], in_=midx_d[:, ph * MCOLS : (ph + 1) * MCOLS]
                )
                acc_t = accp.tile([P, ACCB * DIM], f32, tag="acc")
                n0blk = (sizes[0] // P) if sizes else 0
                if n0blk < ACCB:
                    nc.vector.memset(acc_t[:, n0blk * DIM :], 0.0)
                for k, n in enumerate(sizes):
                    nblk = n // P
                    buf = gbufp.tile([P, max_nblk * DIM], f32, tag="gb")
                    o16 = plan.g16_off[ph][k] - plan.g16_off[ph][0]
                    o128 = plan.g128_off[ph][k] - plan.g128_off[ph][0]
                    nc.gpsimd.dma_gather(
                        out_ap=buf[:, : nblk * DIM].rearrange(
                            "p (j d) -> p j d", d=DIM
                        ),
                        in_ap=x_d[c * CH : (c + 1) * CH, :],
                        idxs_ap=gidx_t[:, o16 : o16 + n // 16],
                        num_idxs=n,
                        num_idxs_reg=n,
                        elem_size=DIM,
                        single_packet=False,
                    )
                    wb = gwgt_t[:, o128 : o128 + nblk].to_broadcast([P, nblk, DIM])
                    bview = buf[:, : nblk * DIM].rearrange("p (j d) -> p j d", d=DIM)
                    aview = acc_t[:, : nblk * DIM].rearrange(
                        "p (j d) -> p j d", d=DIM
                    )
                    if k == 0:
                        nc.vector.tensor_tensor(out=aview, in0=bview, in1=wb, op=mult)
                    else:
                        sc = gbufp.tile([P, max_nblk * DIM], f32, tag="sc")
                        sview = sc[:, : nblk * DIM].rearrange(
                            "p (j d) -> p j d", d=DIM
                        )
                        nc.vector.tensor_tensor(out=sview, in0=bview, in1=wb, op=mult)
                        nc.vector.tensor_tensor(
                            out=aview, in0=aview, in1=sview, op=add
                        )
                minst = nc.gpsimd.dma_scatter_add(
                    out_ap=agg_d[:, :],
                    in_ap=acc_t[:].rearrange("p (j d) -> p j d", d=DIM),
                    idxs_ap=midx_t[:],
                    num_idxs=plan.ACC_SLOTS,
                    num_idxs_reg=HALF,
                    elem_size=DIM,
                    single_packet=False,
                )
                for aw in agg_writes:
                    add_dep_helper(minst.ins, aw.ins, reason="agg init before merge")
                if merge_insts:
                    add_dep_helper(
                        minst.ins, merge_insts[-1].ins, reason="serialize agg RMW"
                    )
                merge_insts.append(minst)

            # final: out = relu((dinv_d*agg) @ W + b) + x
            GB = 4  # blocks per group (512 dst rows)
            n_groups = -(-SHB // GB)
            for g in range(n_groups):
                blks = min(GB, SHB - g * GB)
                ag = finp.tile([P, GB * DIM], f32, tag="ag")
                rinst = nc.sync.dma_start(
                    out=ag[:, : blks * DIM].rearrange("p (bb d) -> p bb d", d=DIM),
                    in_=agg_d[g * GB * P : g * GB * P + blks * P, :].rearrange(
                        "(bb p) d -> p bb d", p=P
                    ),
                )
                for m in merge_insts:
                    add_dep_helper(rinst.ins, m.ins, reason="merge before final read")
                nc.vector.tensor_tensor(
                    out=ag[:, : blks * DIM].rearrange("p (bb d) -> p bb d", d=DIM),
                    in0=ag[:, : blks * DIM].rearrange("p (bb d) -> p bb d", d=DIM),
                    in1=dinvd_t[:, g * GB : g * GB + blks].to_broadcast(
                        [P, blks, DIM]
                    ),
                    op=mult,
                )
                pt = psump.tile([DIM, GB * P], f32, tag="pt")
                for bb in range(blks):
                    nc.tensor.transpose(
                        out=pt[:, bb * P : (bb + 1) * P],
                        in_=ag[:, bb * DIM : (bb + 1) * DIM],
                        identity=ident[:],
                    )
                at = finp.tile([DIM, GB * P], f32, tag="at")
                nc.vector.tensor_copy(out=at[:, : blks * P], in_=pt[:, : blks * P])
                pz = psump.tile([DIM, GB * P], f32, tag="pz")
                nc.tensor.matmul(
                    out=pz[:, : blks * P],
                    lhsT=w_t[:],
                    rhs=at[:, : blks * P],
                    start=True,
                    stop=True,
                )
                zr = finp.tile([DIM, GB * P], f32, tag="zr")
                nc.scalar.activation(
                    out=zr[:, : blks * P],
                    in_=pz[:, : blks * P],
                    func=mybir.ActivationFunctionType.Relu,
                    bias=b_t[:],
                )
                po = psump.tile([P, GB * DIM], f32, tag="po")
                for bb in range(blks):
                    nc.tensor.transpose(
                        out=po[:, bb * DIM : (bb + 1) * DIM],
                        in_=zr[:, bb * P : (bb + 1) * P],
                        identity=ident[:DIM, :DIM],
                    )
                ot = finp.tile([P, GB * DIM], f32, tag="ot")
                nc.vector.tensor_tensor(
                    out=ot[:, : blks * DIM],
                    in0=po[:, : blks * DIM],
                    in1=xs_t[:, g * GB * DIM : (g * GB + blks) * DIM],
                    op=add,
                )
                row0 = g * GB * P
                rows = min(SHARD - row0, blks * P)
                fb2 = rows // P
                if fb2:
                    nc.sync.dma_start(
                        out=out_d[row0 : row0 + fb2 * P, :].rearrange(
                            "(bb p) d -> p bb d", p=P
                        ),
                        in_=ot[:, : fb2 * DIM].rearrange("p (bb d) -> p bb d", d=DIM),
                    )
                rem2 = rows - fb2 * P
                if rem2:
                    nc.sync.dma_start(
                        out=out_d[row0 + fb2 * P : row0 + rows, :],
                        in_=ot[:rem2, fb2 * DIM : (fb2 + 1) * DIM],
                    )

    nc.compile()
    return nc


def run(plan, nc, in_maps, trace=False, tmpdir=None):
    _install_ntff_hook()
    from concourse.bass_utils import run_bass_kernel_spmd

    res = run_bass_kernel_spmd(
        nc,
        in_maps,
        core_ids=list(range(plan.n_cores)),
        trace=trace,
        tmpdir=tmpdir,
    )
    outs = [res.results[i]["out"] for i in range(plan.n_cores)]
    return np.concatenate(outs, axis=0), res


_CACHE = {}


def kernel(x, edge_index, W, b):
    plan, in_maps = preprocess(x, edge_index, W, b)
    sig = tuple(tuple(s) for s in plan.pass_sizes)
    ent = _CACHE.get("prog")
    if ent is None or ent[0] != sig:
        nc = build_program(plan)
        _CACHE["prog"] = (sig, nc)
    nc = _CACHE["prog"][1]
    out, _ = run(plan, nc, in_maps)
    return out


# revision 6
# speedup vs baseline: 2.4400x; 2.4400x over previous
"""GCNConv layer on 8 Trainium2 NeuronCores (Bass/Tile).

Strategy (graph/data parallel, edges partitioned by destination):
  out = relu( D^-1/2 (A+I) D^-1/2 (x W) + b ) + x
      = relu( (dinv_d * (sum_{e->d} dinv_s x_s + dinv_d x_d)) @ W + b ) + x
(using linearity: the W matmul is applied after aggregation).

Each core owns N/8 destination nodes. Per core:
  - source nodes are split into 4 chunks of N/4 rows so gather indices fit
    int16 (dma_gather requirement)
  - per (chunk c, dst-half h): destinations ordered by in-degree from chunk c
    (descending), so the k-th incoming edge of every dst forms a *prefix* of
    the ordering (ELL layout).  Pass k = one dma_gather of the k-th edges'
    source rows + DVE multiply by dinv_src + DVE accumulate.
  - gathers are spread round-robin over the 4 SWDGE queues: each queue's
    descriptor generation runs on its own Q7 core pair, ~4x faster than one.
  - per-phase accumulator is scaled by dinv_d (rank order) and written
    *densely* to a per-chunk HBM table; the final phase re-gathers the four
    permuted contributions per 512-row group (no scatter-add, no RMW chain).
  - final: sum 4 gathered tables + resident self-loop term dinv_d^2 x_d,
    transpose via PE, matmul with W, fused bias+relu on ACT, transpose back,
    add residual x, store.

Edge sets are padded with weight-0 fake edges so all 8 cores run the exact
same static program (SPMD) with per-core data only.
"""

import sys
import types

sys.path.insert(0, "/opt/trn_rl_repo")

import numpy as np

DIM = 64
N_CORES = 8
N_CHUNKS = 4
N_HALVES = 2
N_QUEUES = 4
P = 128
GB = 4  # dst blocks per final-phase group


def _install_ntff_hook():
    """run_bass_kernel_spmd(trace=True) needs antenv.axon_hooks; the image
    lacks it - install an equivalent backed by libaxon_pjrt.so."""
    if "antenv.axon_hooks" in sys.modules:
        return
    try:
        sys.path.insert(0, "/root/.axon_site")
        from trn_agent_boot.trn_boot import _ntff_profile_via_ctypes

        hook = _ntff_profile_via_ctypes("/opt/axon/libaxon_pjrt.so")
    except Exception:
        hook = None
    mod = types.ModuleType("antenv.axon_hooks")
    mod.get_axon_ntff_profile_hook = lambda: hook
    mod.set_axon_ntff_profile_hook = lambda h: None
    sys.modules["antenv.axon_hooks"] = mod


class Plan:
    """Static (core-independent) program structure."""

    def __init__(self, n_nodes, n_cores, n_chunks, n_halves):
        assert n_nodes % n_cores == 0
        assert n_nodes % n_chunks == 0
        self.N = n_nodes
        self.n_cores = n_cores
        self.n_chunks = n_chunks
        self.n_halves = n_halves
        self.SHARD = n_nodes // n_cores          # dst rows per core
        self.CH = n_nodes // n_chunks            # src rows per chunk
        assert self.CH <= 32767, "chunk must fit int16 index"
        assert self.SHARD % n_halves == 0
        self.HALF = self.SHARD // n_halves       # dst rows per phase
        self.SHB = -(-self.SHARD // P)           # shard blocks (ceil)
        self.ACCB = -(-self.HALF // P)           # accumulator blocks
        self.ACC_SLOTS = self.ACCB * P
        assert self.ACC_SLOTS % 16 == 0
        self.n_phases = n_chunks * n_halves
        self.n_groups = -(-self.SHB // GB)
        self.pass_sizes = None   # [phase][k] -> padded slot count (all cores)
        self.g16_off = None      # [phase][k] -> col offset into gidx blob
        self.g128_off = None     # [phase][k] -> col offset into gwgt blob
        self.GCOLS = 0
        self.WCOLS = 0
        # merge-gather idx blob layout: per (group, chunk) a [128, gsz/16]
        self.group_sizes = [
            min(GB, self.SHB - g * GB) * P for g in range(self.n_groups)
        ]
        self.MG_COLS = sum(s // 16 for s in self.group_sizes) * n_chunks

    def mg_off(self, g, c):
        o = 0
        for gg in range(g):
            o += (self.group_sizes[gg] // 16) * self.n_chunks
        return o + (self.group_sizes[g] // 16) * c


def _rep16(vals_i16, n):
    """[n] int -> [128, n//16] wrapped-in-16-partitions, replicated 8x."""
    a = np.asarray(vals_i16, dtype=np.int16).reshape(n // 16, 16).T  # [16, n/16]
    return np.tile(a, (8, 1))


def preprocess(x, edge_index, W, b):
    """Host-side sharding: build the static plan + per-core input maps."""
    x = np.ascontiguousarray(np.asarray(x, dtype=np.float32))
    N = x.shape[0]
    plan = Plan(N, N_CORES, N_CHUNKS, N_HALVES)
    src = np.asarray(edge_index[0], dtype=np.int64)
    dst = np.asarray(edge_index[1], dtype=np.int64)
    deg = np.bincount(dst, minlength=N).astype(np.float64) + 1.0
    dinv = (1.0 / np.sqrt(deg)).astype(np.float32)

    SHARD, CH, HALF = plan.SHARD, plan.CH, plan.HALF
    NPH = plan.n_phases

    core_of = dst // SHARD
    per_core = []  # [core][phase] -> dict(passes=[src arrays], rank)
    for i in range(N_CORES):
        m = core_of == i
        s_i = src[m]
        d_i = dst[m] - i * SHARD
        c_i = s_i // CH
        h_i = d_i // HALF
        phases = []
        for c in range(N_CHUNKS):
            for h in range(N_HALVES):
                mm = (c_i == c) & (h_i == h)
                s = s_i[mm]
                d = d_i[mm] - h * HALF
                deg_ch = np.bincount(d, minlength=HALF)
                order = np.argsort(-deg_ch, kind="stable")  # rank -> dst slot
                rank = np.empty(HALF, dtype=np.int64)
                rank[order] = np.arange(HALF)
                perm = np.argsort(rank[d], kind="stable")
                s_sorted = s[perm]
                counts = deg_ch[order]                  # per rank, descending
                cum = np.concatenate([[0], np.cumsum(counts)])
                K = int(counts[0]) if len(s) else 0
                passes = []
                for k in range(K):
                    L = int(np.searchsorted(-counts, -k, side="left"))
                    pos = cum[:L] + k
                    passes.append(s_sorted[pos])
                phases.append({"passes": passes, "rank": rank})
        per_core.append(phases)

    # static pass structure: max over cores, pad to 128
    pass_sizes = []
    for ph in range(NPH):
        K = max(len(per_core[i][ph]["passes"]) for i in range(N_CORES))
        sizes = []
        for k in range(K):
            L = max(
                len(per_core[i][ph]["passes"][k])
                if k < len(per_core[i][ph]["passes"])
                else 0
                for i in range(N_CORES)
            )
            sizes.append(-(-L // P) * P)
        pass_sizes.append(sizes)
    plan.pass_sizes = pass_sizes

    g16_off, g128_off = [], []
    o16 = o128 = 0
    for ph in range(NPH):
        offs16, offs128 = [], []
        for n in pass_sizes[ph]:
            offs16.append(o16)
            offs128.append(o128)
            o16 += n // 16
            o128 += n // P
        g16_off.append(offs16)
        g128_off.append(offs128)
    plan.g16_off, plan.g128_off = g16_off, g128_off
    plan.GCOLS = max(o16, 16)
    plan.WCOLS = max(o128, 1)

    # per-core blobs
    in_maps = []
    W = np.ascontiguousarray(np.asarray(W, dtype=np.float32))
    b = np.ascontiguousarray(np.asarray(b, dtype=np.float32).reshape(DIM, 1))
    for i in range(N_CORES):
        gidx = np.zeros((P, plan.GCOLS), dtype=np.int16)
        gwgt = np.zeros((P, plan.WCOLS), dtype=np.float32)
        for ph in range(NPH):
            c = ph // N_HALVES
            pdata = per_core[i][ph]
            for k, n in enumerate(pass_sizes[ph]):
                s_pass = (
                    pdata["passes"][k]
                    if k < len(pdata["passes"])
                    else np.empty(0, np.int64)
                )
                L = len(s_pass)
                iv = np.zeros(n, dtype=np.int16)
                wv = np.zeros(n, dtype=np.float32)
                iv[:L] = (s_pass - c * CH).astype(np.int16)
                wv[:L] = dinv[s_pass]
                gidx[:, g16_off[ph][k] : g16_off[ph][k] + n // 16] = _rep16(iv, n)
                gwgt[:, g128_off[ph][k] : g128_off[ph][k] + n // P] = (
                    wv.reshape(n // P, P).T
                )
        # dinv of own dst rows, in rank order per (c,h): used to pre-scale acc
        dinvr = np.zeros((P, NPH * plan.ACCB), dtype=np.float32)
        for ph in range(NPH):
            c, h = ph // N_HALVES, ph % N_HALVES
            rank = per_core[i][ph]["rank"]
            dv = np.zeros(plan.ACC_SLOTS, dtype=np.float32)
            # rank r -> dst slot order[r]; dinv value of that dst
            order = np.empty(HALF, dtype=np.int64)
            order[rank] = np.arange(HALF)
            dv[:HALF] = dinv[i * SHARD + h * HALF + order]
            dinvr[:, ph * plan.ACCB : (ph + 1) * plan.ACCB] = dv.reshape(
                plan.ACCB, P
            ).T
        # merge-gather indices: for final group g, chunk c: row d -> h*ACC_SLOTS+rank
        mgidx = np.zeros((P, plan.MG_COLS), dtype=np.int16)
        for g in range(plan.n_groups):
            gsz = plan.group_sizes[g]
            d = np.arange(g * GB * P, g * GB * P + gsz)
            dc = np.clip(d, 0, SHARD - 1)
            hh = dc // HALF
            for c in range(N_CHUNKS):
                ph = c * N_HALVES
                iv = np.zeros(gsz, dtype=np.int16)
                for h in range(N_HALVES):
                    mh = hh == h
                    rank = per_core[i][ph + h]["rank"]
                    iv[mh] = (h * plan.ACC_SLOTS + rank[dc[mh] - h * HALF]).astype(
                        np.int16
                    )
                iv[d >= SHARD] = 0
                o = plan.mg_off(g, c)
                mgidx[:, o : o + gsz // 16] = _rep16(iv, gsz)
        # self-loop scale dinv^2 in node order, [p,b] = val[b*128+p]
        dv = np.zeros((plan.SHB * P,), dtype=np.float32)
        dv[:SHARD] = dinv[i * SHARD : (i + 1) * SHARD] ** 2
        dinvsq = np.ascontiguousarray(dv.reshape(plan.SHB, P).T)
        xsh = np.ascontiguousarray(x[i * SHARD : (i + 1) * SHARD])
        in_maps.append(
            {
                "x": x,
                "xsh": xsh,
                "w": W,
                "bias": b,
                "dinvsq": dinvsq,
                "dinvr": dinvr,
                "gidx": gidx,
                "gwgt": gwgt,
                "mgidx": mgidx,
            }
        )
    return plan, in_maps


_QPATCHED = [False]


def _patch_queue_aware_dma_lanes():
    """Tile assigns DMA-completion sem lanes (DMASW0-7) round-robin in
    scheduled order, ignoring queue_num.  Two SWDGE queues sharing a lane can
    complete out of order and release waiters early.  Partition the 8 lanes
    so queue q owns lanes {2q, 2q+1}."""
    if _QPATCHED[0]:
        return
    _QPATCHED[0] = True
    from concourse import tile_sem_assignment as tsa
    from concourse import bass_isa, mybir

    orig = tsa.TileClockTick._assign_tick

    def qaware(self, inst):
        if (
            isinstance(inst, tsa.DMAInst)
            and inst.engine == mybir.EngineType.Pool
            and not isinstance(inst, bass_isa.UserSyncedRemoteDMADescs)
        ):
            qn = getattr(inst, "queue_num", 0) or 0
            tog = getattr(self, "_q_toggle", None)
            if tog is None:
                tog = self._q_toggle = {}
            t = tog.get(qn, 0)
            tog[qn] = t ^ 1
            self.next_sw_dma_idx = 2 * qn + t
        return orig(self, inst)

    tsa.TileClockTick._assign_tick = qaware


def build_program(plan):
    from concourse import bacc, mybir
    import concourse.tile as tile
    from concourse.masks import make_identity
    from concourse.tile import add_dep_helper

    _patch_queue_aware_dma_lanes()

    N = plan.N
    SHARD, CH, HALF = plan.SHARD, plan.CH, plan.HALF
    SHB, ACCB = plan.SHB, plan.ACCB
    NPH = plan.n_phases
    FB = SHARD // P              # full shard blocks
    REM = SHARD - FB * P         # partial block rows
    f32 = mybir.dt.float32
    i16 = mybir.dt.int16
    mult = mybir.AluOpType.mult
    add = mybir.AluOpType.add

    nc = bacc.Bacc("TRN2", target_bir_lowering=False, num_swdge_queues=N_QUEUES)
    x_d = nc.dram_tensor("x", [N, DIM], f32, kind="ExternalInput")
    xsh_d = nc.dram_tensor("xsh", [SHARD, DIM], f32, kind="ExternalInput")
    w_d = nc.dram_tensor("w", [DIM, DIM], f32, kind="ExternalInput")
    b_d = nc.dram_tensor("bias", [DIM, 1], f32, kind="ExternalInput")
    dinvsq_d = nc.dram_tensor("dinvsq", [P, SHB], f32, kind="ExternalInput")
    dinvr_d = nc.dram_tensor("dinvr", [P, NPH * ACCB], f32, kind="ExternalInput")
    gidx_d = nc.dram_tensor("gidx", [P, plan.GCOLS], i16, kind="ExternalInput")
    gwgt_d = nc.dram_tensor("gwgt", [P, plan.WCOLS], f32, kind="ExternalInput")
    mgidx_d = nc.dram_tensor("mgidx", [P, plan.MG_COLS], i16, kind="ExternalInput")
    # per-chunk permuted aggregate tables (both halves stacked)
    accd = [
        nc.dram_tensor(f"accd{c}", [N_HALVES * plan.ACC_SLOTS, DIM], f32)
        for c in range(N_CHUNKS)
    ]
    out_d = nc.dram_tensor("out", [SHARD, DIM], f32, kind="ExternalOutput")

    max_nblk = max((max(s) for s in plan.pass_sizes if s), default=P) // P
    qctr = [0]

    def next_q():
        q = qctr[0] % N_QUEUES
        qctr[0] += 1
        return q

    with tile.TileContext(nc) as tc:
        with (
            tc.tile_pool(name="const", bufs=1) as constp,
            tc.tile_pool(name="io", bufs=2) as iop,
            tc.tile_pool(name="gbuf", bufs=6) as gbufp,
            tc.tile_pool(name="accp", bufs=2) as accp,
            tc.tile_pool(name="fin", bufs=3) as finp,
            tc.tile_pool(name="psum", bufs=2, space="PSUM") as psump,
        ):
            ident = constp.tile([P, P], f32)
            make_identity(nc, ident[:])
            w_t = constp.tile([DIM, DIM], f32)
            nc.sync.dma_start(out=w_t[:], in_=w_d[:])
            b_t = constp.tile([DIM, 1], f32)
            nc.sync.dma_start(out=b_t[:], in_=b_d[:])
            dinvsq_t = constp.tile([P, SHB], f32)
            nc.sync.dma_start(out=dinvsq_t[:], in_=dinvsq_d[:])
            dinvr_t = constp.tile([P, NPH * ACCB], f32)
            nc.sync.dma_start(out=dinvr_t[:], in_=dinvr_d[:])
            mgidx_t = constp.tile([P, plan.MG_COLS], i16)
            nc.sync.dma_start(out=mgidx_t[:], in_=mgidx_d[:])

            # x shard resident: [128, SHB*64], row b*128+p -> [p, b*64:(b+1)*64]
            xs_t = constp.tile([P, SHB * DIM], f32)
            if REM:
                nc.vector.memset(xs_t[:, FB * DIM :], 0.0)
            nc.sync.dma_start(
                out=xs_t[:, : FB * DIM].rearrange("p (bb d) -> p bb d", d=DIM),
                in_=xsh_d[0 : FB * P, :].rearrange("(bb p) d -> p bb d", p=P),
            )
            if REM:
                nc.sync.dma_start(
                    out=xs_t[:REM, FB * DIM :],
                    in_=xsh_d[FB * P : SHARD, :],
                )

            # resident self-loop term dinv_d^2 * x_d (node order)
            self_t = constp.tile([P, SHB * DIM], f32)
            nc.vector.tensor_tensor(
                out=self_t[:].rearrange("p (bb d) -> p bb d", d=DIM),
                in0=xs_t[:].rearrange("p (bb d) -> p bb d", d=DIM),
                in1=dinvsq_t[:].to_broadcast([P, SHB, DIM]),
                op=mult,
            )

            # aggregation phases
            accd_writes = []
            for ph in range(NPH):
                c, h = ph // N_HALVES, ph % N_HALVES
                sizes = plan.pass_sizes[ph]
                if sizes:
                    gcols = plan.g16_off[ph][-1] + sizes[-1] // 16 - plan.g16_off[ph][0]
                    wcols = plan.g128_off[ph][-1] + sizes[-1] // P - plan.g128_off[ph][0]
                    gidx_t = iop.tile([P, gcols], i16, tag="gidx")
                    nc.sync.dma_start(
                        out=gidx_t[:],
                        in_=gidx_d[
                            :, plan.g16_off[ph][0] : plan.g16_off[ph][0] + gcols
                        ],
                    )
                    gwgt_t = iop.tile([P, wcols], f32, tag="gwgt")
                    nc.sync.dma_start(
                        out=gwgt_t[:],
                        in_=gwgt_d[
                            :, plan.g128_off[ph][0] : plan.g128_off[ph][0] + wcols
                        ],
                    )
                acc_t = accp.tile([P, ACCB * DIM], f32, tag="acc")
                n0blk = (sizes[0] // P) if sizes else 0
                if n0blk < ACCB:
                    nc.vector.memset(acc_t[:, n0blk * DIM :], 0.0)
                for k, n in enumerate(sizes):
                    nblk = n // P
                    buf = gbufp.tile([P, max_nblk * DIM], f32, tag="gb")
                    o16 = plan.g16_off[ph][k] - plan.g16_off[ph][0]
                    o128 = plan.g128_off[ph][k] - plan.g128_off[ph][0]
                    nc.gpsimd.dma_gather(
                        out_ap=buf[:, : nblk * DIM].rearrange(
                            "p (j d) -> p j d", d=DIM
                        ),
                        in_ap=x_d[c * CH : (c + 1) * CH, :],
                        idxs_ap=gidx_t[:, o16 : o16 + n // 16],
                        num_idxs=n,
                        num_idxs_reg=n,
                        elem_size=DIM,
                        single_packet=False,
                        queue_num=next_q(),
                    )
                    wb = gwgt_t[:, o128 : o128 + nblk].to_broadcast([P, nblk, DIM])
                    bview = buf[:, : nblk * DIM].rearrange("p (j d) -> p j d", d=DIM)
                    aview = acc_t[:, : nblk * DIM].rearrange(
                        "p (j d) -> p j d", d=DIM
                    )
                    if k == 0:
                        nc.vector.tensor_tensor(out=aview, in0=bview, in1=wb, op=mult)
                    else:
                        nc.vector.tensor_tensor(out=bview, in0=bview, in1=wb, op=mult)
                        nc.vector.tensor_tensor(
                            out=aview, in0=aview, in1=bview, op=add
                        )
                # pre-scale by dinv_d (rank order) and write densely to HBM
                nc.vector.tensor_tensor(
                    out=acc_t[:].rearrange("p (j d) -> p j d", d=DIM),
                    in0=acc_t[:].rearrange("p (j d) -> p j d", d=DIM),
                    in1=dinvr_t[:, ph * ACCB : (ph + 1) * ACCB].to_broadcast(
                        [P, ACCB, DIM]
                    ),
                    op=mult,
                )
                winst = nc.sync.dma_start(
                    out=accd[c][
                        h * plan.ACC_SLOTS : (h + 1) * plan.ACC_SLOTS, :
                    ].rearrange("(j p) d -> p j d", p=P),
                    in_=acc_t[:].rearrange("p (j d) -> p j d", d=DIM),
                )
                accd_writes.append((c, winst))

            # final: out = relu((sum_c perm_c(accd_c) + self) @ W + b) + x
            for g in range(plan.n_groups):
                gsz = plan.group_sizes[g]
                blks = gsz // P
                mg = []
                for c in range(N_CHUNKS):
                    mb = finp.tile([P, GB * DIM], f32, tag=f"mg{c}")
                    o = plan.mg_off(g, c)
                    ginst = nc.gpsimd.dma_gather(
                        out_ap=mb[:, : blks * DIM].rearrange(
                            "p (j d) -> p j d", d=DIM
                        ),
                        in_ap=accd[c][:, :],
                        idxs_ap=mgidx_t[:, o : o + gsz // 16],
                        num_idxs=gsz,
                        num_idxs_reg=gsz,
                        elem_size=DIM,
                        single_packet=False,
                        queue_num=next_q(),
                    )
                    for cc, wi in accd_writes:
                        if cc == c:
                            add_dep_helper(
                                ginst.ins, wi.ins, reason="accd write before merge"
                            )
                    mg.append(mb)
                t01 = finp.tile([P, GB * DIM], f32, tag="t01")
                nc.vector.tensor_tensor(
                    out=t01[:, : blks * DIM],
                    in0=mg[0][:, : blks * DIM],
                    in1=mg[1][:, : blks * DIM],
                    op=add,
                )
                t23 = finp.tile([P, GB * DIM], f32, tag="t23")
                nc.vector.tensor_tensor(
                    out=t23[:, : blks * DIM],
                    in0=mg[2][:, : blks * DIM],
                    in1=mg[3][:, : blks * DIM],
                    op=add,
                )
                ag = finp.tile([P, GB * DIM], f32, tag="ag")
                nc.vector.tensor_tensor(
                    out=ag[:, : blks * DIM],
                    in0=t01[:, : blks * DIM],
                    in1=t23[:, : blks * DIM],
                    op=add,
                )
                nc.vector.tensor_tensor(
                    out=ag[:, : blks * DIM],
                    in0=ag[:, : blks * DIM],
                    in1=self_t[:, g * GB * DIM : (g * GB + blks) * DIM],
                    op=add,
                )
                pt = psump.tile([DIM, GB * P], f32, tag="pt")
                for bb in range(blks):
                    nc.tensor.transpose(
                        out=pt[:, bb * P : (bb + 1) * P],
                        in_=ag[:, bb * DIM : (bb + 1) * DIM],
                        identity=ident[:],
                    )
                at = finp.tile([DIM, GB * P], f32, tag="at")
                nc.vector.tensor_copy(out=at[:, : blks * P], in_=pt[:, : blks * P])
                pz = psump.tile([DIM, GB * P], f32, tag="pz")
                nc.tensor.matmul(
                    out=pz[:, : blks * P],
                    lhsT=w_t[:],
                    rhs=at[:, : blks * P],
                    start=True,
                    stop=True,
                )
                zr = finp.tile([DIM, GB * P], f32, tag="zr")
                nc.scalar.activation(
                    out=zr[:, : blks * P],
                    in_=pz[:, : blks * P],
                    func=mybir.ActivationFunctionType.Relu,
                    bias=b_t[:],
                )
                po = psump.tile([P, GB * DIM], f32, tag="po")
                for bb in range(blks):
                    nc.tensor.transpose(
                        out=po[:, bb * DIM : (bb + 1) * DIM],
                        in_=zr[:, bb * P : (bb + 1) * P],
                        identity=ident[:DIM, :DIM],
                    )
                ot = finp.tile([P, GB * DIM], f32, tag="ot")
                nc.vector.tensor_tensor(
                    out=ot[:, : blks * DIM],
                    in0=po[:, : blks * DIM],
                    in1=xs_t[:, g * GB * DIM : (g * GB + blks) * DIM],
                    op=add,
                )
                row0 = g * GB * P
                rows = min(SHARD - row0, blks * P)
                fb2 = rows // P
                if fb2:
                    nc.sync.dma_start(
                        out=out_d[row0 : row0 + fb2 * P, :].rearrange(
                            "(bb p) d -> p bb d", p=P
                        ),
                        in_=ot[:, : fb2 * DIM].rearrange("p (bb d) -> p bb d", d=DIM),
                    )
                rem2 = rows - fb2 * P
                if rem2:
                    nc.sync.dma_start(
                        out=out_d[row0 + fb2 * P : row0 + rows, :],
                        in_=ot[:rem2, fb2 * DIM : (fb2 + 1) * DIM],
                    )

    nc.compile()
    return nc


def run(plan, nc, in_maps, trace=False, tmpdir=None):
    _install_ntff_hook()
    from concourse.bass_utils import run_bass_kernel_spmd

    res = run_bass_kernel_spmd(
        nc,
        in_maps,
        core_ids=list(range(plan.n_cores)),
        trace=trace,
        tmpdir=tmpdir,
    )
    outs = [res.results[i]["out"] for i in range(plan.n_cores)]
    return np.concatenate(outs, axis=0), res


_CACHE = {}


def kernel(x, edge_index, W, b):
    plan, in_maps = preprocess(x, edge_index, W, b)
    sig = tuple(tuple(s) for s in plan.pass_sizes)
    ent = _CACHE.get("prog")
    if ent is None or ent[0] != sig:
        nc = build_program(plan)
        _CACHE["prog"] = (sig, nc)
    nc = _CACHE["prog"][1]
    out, _ = run(plan, nc, in_maps)
    return out
